# revision 10
# baseline (speedup 1.0000x reference)
"""Trainium2 Bass kernel for causal multi-head attention with RoPE.

Problem: x[2,2048,2048], 16 heads, head_dim 128, fp32.
  q/k/v = x @ w{q,k,v}^T ; RoPE on q,k ; causal softmax(q k^T / sqrt(128)) @ v ; out @ wo^T

Sharding: Megatron tensor-parallel over heads - 2 heads per core on 8 cores.
Each core computes a partial y (its 2 heads' contribution through wo); the host
sums the 8 partials.  No device collectives.

Per-core design (v2, all matmul operands bf16; fp8 was tested on CPU and
fails the 2e-2 gate at ~4e-2):
  - x pre-transposed/tiled bf16 on host; q^T,k^T computed feature-major,
    v token-major.  RoPE rotate-half built with a tiny constant matmul on
    the PE (prot = R^T q) so the DVE does only 3 tensor_tensor ops per
    RoPE application instead of 5.
  - scores computed transposed S^T[key,q] = kT.T @ qT, one K=128 pass.
    Causal handled at 128-granularity: for the 4 diagonal-crossing key
    tiles the query slice is trimmed to [128*mi : 512], which makes the
    score/exp/AV work exactly the lower-triangular block count; the
    remaining triangle uses a single [128,512] 0/1 bf16 mask (prefix
    slices of it serve every trim width).
  - softmax without max-subtraction (scores bounded, exp safe in fp32):
    P^T = exp(S^T/sqrt(128)) on ACT, bf16.
  - row sums: P tiles are accumulated into an f32 SBUF tile on the Pool
    engine (which is otherwise idle); one ones[128,128]-stationary matmul
    per (qt,h) then yields the per-query sums replicated across all 128
    PSUM partitions, so 1/r comes from one fast [128,512]
    reciprocal_approx_fast and feeds a plain tensor_tensor multiply - no
    partition_broadcast, no slow single-partition reciprocal.
  - o^T = v.T @ P^T accumulated in PSUM; normalization deferred by one
    half-unit so the PE never waits on the Pool accumulation.
  - y rows = (o_norm^T).T @ woT written bf16 (host sums partials in
    fp64); PSUM->SBUF y copies alternate ACT/DVE to balance engines.
  - phase interleaving: attention of (b,qt) is emitted as soon as its
    token tiles are projected, filling the projection-phase gaps.
"""

import math
import sys

sys.path.insert(0, "/opt/trn_rl_repo")

import ml_dtypes  # noqa: E402
import numpy as np  # noqa: E402

P = 128
D = 2048
HD = 128  # head dim
B = 2
T = 2048
TOK = B * T  # 4096
NCORES = 8
HPC = 2  # heads per core
DC = HPC * HD  # 256 dims per core
CCHUNKS = D // P  # 16 contraction chunks
CPAIRS = CCHUNKS // 2  # 8 chunk pairs (one DMA each)
TT = TOK // 512  # 8 token tiles of 512
QT = T // 512  # 4 query tiles per batch
KT_PER_Q = 512 // P  # 4 key tiles per query tile

_CACHE = {}


def _build_nc():
    import concourse.bacc as bacc
    import concourse.mybir as mybir
    import concourse.tile as tile

    f32 = mybir.dt.float32
    f32r = mybir.dt.float32r
    bf16 = mybir.dt.bfloat16

    nc = bacc.Bacc("TRN2", target_bir_lowering=False, debug=False, num_devices=NCORES)

    # x pre-tiled on host: [tt, cpair, 128, 2, 512] bf16, contiguous per pair
    xTt = nc.dram_tensor("xTt", [TT, CPAIRS, P, 2, 512], bf16,
                         kind="ExternalInput").ap()
    cosT = nc.dram_tensor("cosT", [HD, TOK], bf16, kind="ExternalInput").ap()
    sinT = nc.dram_tensor("sinT", [HD, TOK], bf16, kind="ExternalInput").ap()
    wqT = nc.dram_tensor("wqT", [D, DC], bf16, kind="ExternalInput").ap()
    wkT = nc.dram_tensor("wkT", [D, DC], bf16, kind="ExternalInput").ap()
    wvT = nc.dram_tensor("wvT", [D, DC], bf16, kind="ExternalInput").ap()
    woT = nc.dram_tensor("woT", [DC, D], bf16, kind="ExternalInput").ap()
    rotM = nc.dram_tensor("rotM", [HD, HD], bf16, kind="ExternalInput").ap()
    y = nc.dram_tensor("y", [TOK, D], bf16, kind="ExternalOutput").ap()

    inv_sqrt_hd = 1.0 / math.sqrt(HD)

    with tile.TileContext(nc) as tc:
        with (
            tc.tile_pool(name="consts", bufs=1) as consts,
            tc.tile_pool(name="wpool", bufs=1) as wpool,
            tc.tile_pool(name="qkv", bufs=1) as qkv,
            tc.tile_pool(name="xp", bufs=4) as xp,
            tc.tile_pool(name="csp", bufs=2) as csp,
            tc.tile_pool(name="ropep", bufs=2) as ropep,
            tc.tile_pool(name="ptp", bufs=4) as ptp,
            tc.tile_pool(name="pap", bufs=4) as pap,
            tc.tile_pool(name="rrp", bufs=2) as rrp,
            tc.tile_pool(name="onp", bufs=3) as onp,
            tc.tile_pool(name="ysp", bufs=3) as ysp,
            tc.tile_pool(name="ps", bufs=8, space="PSUM") as ps,
        ):
            # ---- constants ----
            # single causal 0/1 bf16 mask: keep where q_local - key_local >= 0.
            # Diagonal tile mi uses mask[:, :512-128*mi] against the trimmed
            # query slice starting at 128*mi.
            mask = consts.tile([P, 512], bf16, tag="mask")
            nc.gpsimd.memset(mask[:], 1.0)
            nc.gpsimd.affine_select(
                out=mask[:], in_=mask[:], compare_op=mybir.AluOpType.is_ge,
                fill=0.0, base=0, channel_multiplier=-1, pattern=[[1, 512]],
            )
            ones_sq = consts.tile([P, P], bf16, tag="ones_sq")
            nc.gpsimd.memset(ones_sq[:], 1.0)
            rot_t = consts.tile([P, HD], bf16, tag="rot")
            nc.sync.dma_start(rot_t[:], rotM)

            # ---- resident weights (DMAs staggered into tile 0's loop) ----
            wq_t = wpool.tile([P, CCHUNKS, DC], bf16, tag="wq")
            wk_t = wpool.tile([P, CCHUNKS, DC], bf16, tag="wk")
            wv_t = wpool.tile([P, CCHUNKS, DC], bf16, tag="wv")
            wo_t = wpool.tile([P, HPC, D], bf16, tag="wo")

            def emit_w_pair(cp):
                csl = slice(2 * cp, 2 * cp + 2)
                for wt, wdram in ((wq_t, wqT), (wk_t, wkT), (wv_t, wvT)):
                    nc.sync.dma_start(
                        wt[:, csl, :],
                        wdram.rearrange("(co ci) d -> ci co d", ci=P)[:, csl, :])

            # ---- resident activations ----
            qT_t = qkv.tile([P, HPC, TOK], bf16, tag="qT")  # [head_dim, h, tok]
            kT_t = qkv.tile([P, HPC, TOK], bf16, tag="kT")
            v_t = qkv.tile([P, TOK // P, DC], bf16, tag="v")  # [tok%128, blk, d]

            # ---- phase 1 tile body: projections + RoPE ----
            def emit_tile(tt):
                tsl = slice(tt * 512, (tt + 1) * 512)
                cos_t = csp.tile([P, 512], bf16, tag="cos")
                nc.scalar.dma_start(cos_t[:], cosT[:, tsl])
                sin_t = csp.tile([P, 512], bf16, tag="sin")
                nc.scalar.dma_start(sin_t[:], sinT[:, tsl])

                pq = [ps.tile([P, 512], f32, tag="ps", name=f"pq{i}") for i in range(HPC)]
                pk = [ps.tile([P, 512], f32, tag="ps", name=f"pk{i}") for i in range(HPC)]
                # two banks hold all four v accumulators ([t128, 256] pairs
                # side by side); see start/skip_group_check notes below.
                pv = [ps.tile([P, 512], f32, tag="ps", name=f"pv{i}") for i in range(2)]

                for cp in range(CPAIRS):
                    if tt == 0 and cp == 0:
                        emit_w_pair(0)
                        emit_w_pair(1)
                    if tt == 0 and cp + 2 < CPAIRS:
                        emit_w_pair(cp + 2)
                    xt = xp.tile([P, 2, 512], bf16, tag="x")
                    nc.sync.dma_start(xt[:], xTt[tt, cp])
                    for j in range(2):
                        c = 2 * cp + j
                        st, sp = (c == 0), (c == CCHUNKS - 1)
                        xj = xt[:, j, :]
                        for h in range(HPC):
                            dsl = slice(h * HD, (h + 1) * HD)
                            nc.tensor.matmul(pq[h][:], wq_t[:, c, dsl], xj,
                                             start=st, stop=sp)
                            nc.tensor.matmul(pk[h][:], wk_t[:, c, dsl], xj,
                                             start=st, stop=sp)
                        for s4 in range(4):
                            half = s4 % 2
                            nc.tensor.matmul(
                                pv[s4 // 2][:, half * DC:(half + 1) * DC],
                                xt[:, j, s4 * P:(s4 + 1) * P],
                                wv_t[:, c, :],
                                start=st and half == 0, stop=sp,
                                skip_group_check=half == 1)

                # evacuate PSUM: q/k/v raw copies on ACT
                for h in range(HPC):
                    nc.scalar.copy(qT_t[:, h, tsl], pq[h][:])
                    nc.scalar.copy(kT_t[:, h, tsl], pk[h][:])
                for s4 in range(4):
                    half = s4 % 2
                    nc.scalar.copy(v_t[:, tt * 4 + s4, :],
                                   pv[s4 // 2][:, half * DC:(half + 1) * DC])
                # RoPE in place: rot-half via PE (prot = rotM^T @ raw), then
                # dst = raw*cos + prot*sin with 3 DVE tensor_tensor ops.
                for dst_t in (qT_t, kT_t):
                    for h in range(HPC):
                        dst = dst_t[:, h, tsl]
                        prot = ps.tile([P, 512], f32, tag="ps", name="prot")
                        nc.tensor.matmul(prot[:], rot_t[:], dst,
                                         start=True, stop=True)
                        nc.vector.tensor_mul(out=dst, in0=dst, in1=cos_t[:])
                        rtmp = ropep.tile([P, 512], bf16, tag="rtmp")
                        nc.vector.tensor_mul(out=rtmp[:], in0=prot[:], in1=sin_t[:])
                        nc.vector.tensor_add(out=dst, in0=dst, in1=rtmp[:])

            # ---- phase 2: attention + output projection ----
            pending_norm = []
            pending_y = []

            def emit_norm(pacc, onorm, h):
                # ones^T @ pacc -> per-query row sums replicated on all 128
                # partitions; fast approx reciprocal; normalize o in place.
                # (pacc accumulated in f32 on Pool, rounded once to bf16 so the
                # matmul runs in the 16-bit path; ~0.2% on the denominator.)
                pacc_bf = pap.tile([P, 512], bf16, tag="pacc_bf")
                nc.gpsimd.tensor_copy(pacc_bf[:], pacc[:])
                pr = ps.tile([P, 512], f32, tag="ps", name="pr")
                nc.tensor.matmul(pr[:], ones_sq[:], pacc_bf[:],
                                 start=True, stop=True)
                rr = rrp.tile([P, 512], f32, tag="rr")
                nc.vector.reciprocal_approx_fast(out=rr[:], in_=pr[:])
                nc.vector.tensor_mul(out=onorm[:, h, :], in0=onorm[:, h, :],
                                     in1=rr[:])

            def emit_yproj(onorm, b, qt):
                for s4 in range(4):
                    r0 = b * T + qt * 512 + s4 * P
                    ystage = ysp.tile([P, D], bf16, tag="ystage")
                    for dout in range(4):
                        py = ps.tile([P, 512], f32, tag="ps", name="py")
                        for h in range(HPC):
                            nc.tensor.matmul(
                                py[:],
                                onorm[:, h, s4 * P:(s4 + 1) * P],
                                wo_t[:, h, dout * 512:(dout + 1) * 512],
                                start=(h == 0), stop=(h == HPC - 1))
                        if dout % 2 == 0:
                            nc.scalar.copy(ystage[:, dout * 512:(dout + 1) * 512],
                                           py[:])
                        else:
                            nc.vector.tensor_copy(
                                ystage[:, dout * 512:(dout + 1) * 512], py[:])
                    nc.sync.dma_start(y[r0:r0 + P, :], ystage[:])

            def emit_attn(b, qt):
                qbase = b * T + qt * 512
                nkt = KT_PER_Q * (qt + 1)
                onorm = onp.tile([P, HPC, 512], bf16, tag="onorm")
                for h in range(HPC):
                    po = ps.tile([P, 512], f32, tag="ps", name="po")
                    pacc = pap.tile([P, 512], f32, tag="pacc")

                    def emit_score(kt, h=h):
                        mi = kt - KT_PER_Q * qt  # >=0 on the diagonal
                        q0 = P * mi if mi > 0 else 0
                        free = 512 - q0
                        ksl = slice(b * T + kt * P, b * T + (kt + 1) * P)
                        pscore = ps.tile([P, 512], f32, tag="ps", name="pscore")
                        nc.tensor.matmul(pscore[:, :free], kT_t[:, h, ksl],
                                         qT_t[:, h, qbase + q0:qbase + 512],
                                         start=True, stop=True)
                        ptile = ptp.tile([P, 512], bf16, tag="pt", name="ptile")
                        nc.scalar.activation(ptile[:, :free], pscore[:, :free],
                                             mybir.ActivationFunctionType.Exp,
                                             scale=inv_sqrt_hd)
                        if mi >= 0:
                            nc.gpsimd.tensor_mul(out=ptile[:, :free],
                                                 in0=ptile[:, :free],
                                                 in1=mask[:, :free])
                        return ptile, q0, free

                    # kt loop pipelined by one so the PE always has wait-free
                    # score work while ACT runs exp.
                    tiles = {0: emit_score(0)}
                    for kt in range(nkt):
                        if kt + 1 < nkt:
                            tiles[kt + 1] = emit_score(kt + 1)
                        ptile, q0, free = tiles.pop(kt)
                        st, sp = (kt == 0), (kt == nkt - 1)
                        nc.tensor.matmul(po[:, q0:512],
                                         v_t[:, b * (T // P) + kt,
                                             h * HD:(h + 1) * HD],
                                         ptile[:, :free], start=st, stop=sp)
                        # accumulate P on the Pool engine for the row sums
                        if kt == 0:
                            nc.gpsimd.tensor_copy(pacc[:], ptile[:])
                        else:
                            nc.gpsimd.tensor_add(out=pacc[:, q0:512],
                                                 in0=pacc[:, q0:512],
                                                 in1=ptile[:, :free])
                    # o out of PSUM right away; normalization is deferred one
                    # half-unit so the PE never waits on the Pool adds.
                    nc.scalar.copy(onorm[:, h, :], po[:])
                    pending_norm.append((pacc, onorm, h))
                    if len(pending_norm) > 1:
                        emit_norm(*pending_norm.pop(0))

                pending_y.append((onorm, b, qt))
                if len(pending_y) > 2:
                    emit_yproj(*pending_y.pop(0))

            # ---- schedule: interleave attention between projection tiles ----
            for tt in range(TT):
                emit_tile(tt)
                if tt == 1:
                    for h in range(HPC):
                        nc.scalar.dma_start(
                            wo_t[:, h, :],
                            woT.rearrange("(ko ki) n -> ki ko n", ki=P)[:, h, :])
                # attention unit (b,qt) is ready once tiles 0..(b*4+qt) exist
                if tt >= 1:
                    b, qt = divmod(tt - 1, QT)
                    emit_attn(b, qt)
            emit_attn(1, 3)
            for args in pending_norm:
                emit_norm(*args)
            for args in pending_y:
                emit_yproj(*args)

    nc.compile()
    return nc


def get_nc():
    if "nc" not in _CACHE:
        _CACHE["nc"] = _build_nc()
    return _CACHE["nc"]


def make_in_maps(x, cos, sin, wq, wk, wv, wo):
    bf = ml_dtypes.bfloat16
    xT = x.reshape(TOK, D).T  # [D, TOK]
    # [D, TOK] -> [TT, cpair, ci, j, 512]
    xTt = np.ascontiguousarray(
        xT.reshape(CPAIRS, 2, P, TT, 512).transpose(3, 0, 2, 1, 4)).astype(bf)
    cosT = np.ascontiguousarray(cos.reshape(TOK, HD).T).astype(bf)
    sinT = np.ascontiguousarray(sin.reshape(TOK, HD).T).astype(bf)
    # rotate-half as a stationary matmul operand: rot(q)[i] = sum_j R[j,i] q[j]
    rotM = np.zeros((HD, HD), dtype=np.float32)
    for i in range(64):
        rotM[i + 64, i] = -1.0
        rotM[i, i + 64] = 1.0
    rotM = rotM.astype(bf)
    in_maps = []
    for c in range(NCORES):
        dsl = slice(c * DC, (c + 1) * DC)
        in_maps.append({
            "xTt": xTt,
            "cosT": cosT,
            "sinT": sinT,
            "wqT": np.ascontiguousarray(wq[dsl, :].T).astype(bf),
            "wkT": np.ascontiguousarray(wk[dsl, :].T).astype(bf),
            "wvT": np.ascontiguousarray(wv[dsl, :].T).astype(bf),
            "woT": np.ascontiguousarray(wo[:, dsl].T).astype(bf),
            "rotM": rotM,
        })
    return in_maps


def kernel(x, cos, sin, wq, wk, wv, wo):
    from concourse.bass_utils import run_bass_kernel_spmd

    nc = get_nc()
    in_maps = make_in_maps(
        np.asarray(x, dtype=np.float32), np.asarray(cos, dtype=np.float32),
        np.asarray(sin, dtype=np.float32), np.asarray(wq, dtype=np.float32),
        np.asarray(wk, dtype=np.float32), np.asarray(wv, dtype=np.float32),
        np.asarray(wo, dtype=np.float32))
    res = run_bass_kernel_spmd(nc, in_maps, list(range(NCORES)))
    out = np.zeros((TOK, D), dtype=np.float64)
    for m in res.results:
        out += m["y"].astype(np.float64)
    return out.astype(np.float32).reshape(B, T, D)


# revision 21
# speedup vs baseline: 1.6006x; 1.6006x over previous
"""Trainium2 Bass kernel for causal multi-head attention with RoPE.

Problem: x[2,2048,2048], 16 heads, head_dim 128, fp32.
  q/k/v = x @ w{q,k,v}^T ; RoPE on q,k ; causal softmax(q k^T / sqrt(128)) @ v ; out @ wo^T

Sharding: Megatron tensor-parallel over heads - 2 heads per core on 8 cores.
Each core computes a partial y (its 2 heads' contribution through wo); the host
sums the 8 partials.  No device collectives.

Per-core design (v2, all matmul operands bf16; fp8 was tested on CPU and
fails the 2e-2 gate at ~4e-2):
  - x pre-transposed/tiled bf16 on host; q^T,k^T computed feature-major,
    v token-major.  RoPE rotate-half built with a tiny constant matmul on
    the PE (prot = R^T q) so the DVE does only 3 tensor_tensor ops per
    RoPE application instead of 5.
  - scores computed transposed S^T[key,q] = kT.T @ qT, one K=128 pass.
    Causal handled at 128-granularity: for the 4 diagonal-crossing key
    tiles the query slice is trimmed to [128*mi : 512], which makes the
    score/exp/AV work exactly the lower-triangular block count; the
    remaining triangle uses a single [128,512] 0/1 bf16 mask (prefix
    slices of it serve every trim width).
  - softmax without max-subtraction (scores bounded, exp safe in fp32):
    P^T = exp(S^T/sqrt(128)) on ACT, bf16.
  - row sums: P tiles are accumulated into an f32 SBUF tile on the Pool
    engine (which is otherwise idle); one ones[128,128]-stationary matmul
    per (qt,h) then yields the per-query sums replicated across all 128
    PSUM partitions, so 1/r comes from one fast [128,512]
    reciprocal_approx_fast and feeds a plain tensor_tensor multiply - no
    partition_broadcast, no slow single-partition reciprocal.
  - o^T = v.T @ P^T accumulated in PSUM; normalization deferred by one
    half-unit so the PE never waits on the Pool accumulation.
  - y rows = (o_norm^T).T @ woT written bf16 (host sums partials in
    fp64); PSUM->SBUF y copies alternate ACT/DVE to balance engines.
  - phase interleaving: attention of (b,qt) is emitted as soon as its
    token tiles are projected, filling the projection-phase gaps.
"""

import math
import sys

sys.path.insert(0, "/opt/trn_rl_repo")

import ml_dtypes  # noqa: E402
import numpy as np  # noqa: E402

P = 128
D = 2048
HD = 128  # head dim
B = 2
T = 2048
TOK = B * T  # 4096
NCORES = 8
HPC = 2  # heads per core
DC = HPC * HD  # 256 dims per core
CCHUNKS = D // P  # 16 contraction chunks
CPAIRS = CCHUNKS // 2  # 8 chunk pairs (one DMA each)
TT = TOK // 512  # 8 token tiles of 512
QT = T // 512  # 4 query tiles per batch
KT_PER_Q = 512 // P  # 4 key tiles per query tile

_CACHE = {}


def _build_nc():
    import concourse.bacc as bacc
    import concourse.mybir as mybir
    import concourse.tile as tile

    f32 = mybir.dt.float32
    f32r = mybir.dt.float32r
    bf16 = mybir.dt.bfloat16

    nc = bacc.Bacc("TRN2", target_bir_lowering=False, debug=False, num_devices=NCORES)

    # x pre-tiled on host: [tt, cpair, 128, 2, 512] bf16, contiguous per pair
    xTt = nc.dram_tensor("xTt", [TT, CPAIRS, P, 2, 512], bf16,
                         kind="ExternalInput").ap()
    cosT = nc.dram_tensor("cosT", [HD, TOK], bf16, kind="ExternalInput").ap()
    sinT = nc.dram_tensor("sinT", [HD, TOK], bf16, kind="ExternalInput").ap()
    wqT = nc.dram_tensor("wqT", [D, DC], bf16, kind="ExternalInput").ap()
    wkT = nc.dram_tensor("wkT", [D, DC], bf16, kind="ExternalInput").ap()
    wvT = nc.dram_tensor("wvT", [D, DC], bf16, kind="ExternalInput").ap()
    woT = nc.dram_tensor("woT", [DC, D], bf16, kind="ExternalInput").ap()
    y = nc.dram_tensor("y", [TOK, D], bf16, kind="ExternalOutput").ap()

    inv_sqrt_hd = 1.0 / math.sqrt(HD)

    with tile.TileContext(nc) as tc:
        with (
            tc.tile_pool(name="consts", bufs=1) as consts,
            tc.tile_pool(name="wpool", bufs=1) as wpool,
            tc.tile_pool(name="qkv", bufs=1) as qkv,
            tc.tile_pool(name="xp", bufs=4) as xp,
            tc.tile_pool(name="csp", bufs=2) as csp,
            tc.tile_pool(name="ropep", bufs=2) as ropep,
            tc.tile_pool(name="ptp", bufs=4) as ptp,
            tc.tile_pool(name="rrp", bufs=2) as rrp,
            tc.tile_pool(name="onp", bufs=3) as onp,
            tc.tile_pool(name="ysp", bufs=3) as ysp,
            tc.tile_pool(name="ps", bufs=8, space="PSUM") as ps,
        ):
            # ---- constants ----
            # single causal 0/1 bf16 mask: keep where q_local - key_local >= 0.
            # Diagonal tile mi uses mask[:, :512-128*mi] against the trimmed
            # query slice starting at 128*mi.
            mask = consts.tile([P, 512], bf16, tag="mask")
            nc.gpsimd.memset(mask[:], 1.0)
            nc.gpsimd.affine_select(
                out=mask[:], in_=mask[:], compare_op=mybir.AluOpType.is_ge,
                fill=0.0, base=0, channel_multiplier=-1, pattern=[[1, 512]],
            )
            ones_sq = consts.tile([P, P], bf16, tag="ones_sq")
            nc.gpsimd.memset(ones_sq[:], 1.0)

            # ---- resident weights (DMAs staggered into tile 0's loop) ----
            wq_t = wpool.tile([P, CCHUNKS, DC], bf16, tag="wq")
            wk_t = wpool.tile([P, CCHUNKS, DC], bf16, tag="wk")
            wv_t = wpool.tile([P, CCHUNKS, DC], bf16, tag="wv")
            wo_t = wpool.tile([P, HPC, D], bf16, tag="wo")

            def emit_w_pair(cp):
                csl = slice(2 * cp, 2 * cp + 2)
                for wt, wdram in ((wq_t, wqT), (wk_t, wkT), (wv_t, wvT)):
                    nc.sync.dma_start(
                        wt[:, csl, :],
                        wdram.rearrange("(co ci) d -> ci co d", ci=P)[:, csl, :])

            # ---- resident activations ----
            qT_t = qkv.tile([P, HPC, TOK], bf16, tag="qT")  # [head_dim, h, tok]
            kT_t = qkv.tile([P, HPC, TOK], bf16, tag="kT")
            v_t = qkv.tile([P, TOK // P, DC], bf16, tag="v")  # [tok%128, blk, d]

            # ---- phase 1 tile body: projections + RoPE ----
            def emit_tile(tt):
                tsl = slice(tt * 512, (tt + 1) * 512)
                cos_t = csp.tile([P, 512], bf16, tag="cos")
                nc.scalar.dma_start(cos_t[:], cosT[:, tsl])
                sin_t = csp.tile([P, 512], bf16, tag="sin")
                nc.scalar.dma_start(sin_t[:], sinT[:, tsl])

                pq = [ps.tile([P, 512], f32, tag="ps", name=f"pq{i}") for i in range(HPC)]
                pk = [ps.tile([P, 512], f32, tag="ps", name=f"pk{i}") for i in range(HPC)]
                # two banks hold all four v accumulators ([t128, 256] pairs
                # side by side); see start/skip_group_check notes below.
                pv = [ps.tile([P, 512], f32, tag="ps", name=f"pv{i}") for i in range(2)]

                for cp in range(CPAIRS):
                    if tt == 0 and cp == 0:
                        emit_w_pair(0)
                        emit_w_pair(1)
                    if tt == 0 and cp + 2 < CPAIRS:
                        emit_w_pair(cp + 2)
                    xt = xp.tile([P, 2, 512], bf16, tag="x")
                    nc.sync.dma_start(xt[:], xTt[tt, cp])
                    for j in range(2):
                        c = 2 * cp + j
                        st, sp = (c == 0), (c == CCHUNKS - 1)
                        xj = xt[:, j, :]
                        for h in range(HPC):
                            dsl = slice(h * HD, (h + 1) * HD)
                            nc.tensor.matmul(pq[h][:], wq_t[:, c, dsl], xj,
                                             start=st, stop=sp)
                            nc.tensor.matmul(pk[h][:], wk_t[:, c, dsl], xj,
                                             start=st, stop=sp)
                        for s4 in range(4):
                            half = s4 % 2
                            nc.tensor.matmul(
                                pv[s4 // 2][:, half * DC:(half + 1) * DC],
                                xt[:, j, s4 * P:(s4 + 1) * P],
                                wv_t[:, c, :],
                                start=st and half == 0, stop=sp,
                                skip_group_check=half == 1)

                # evacuate PSUM: q/k/v raw copies on ACT
                for h in range(HPC):
                    nc.scalar.copy(qT_t[:, h, tsl], pq[h][:])
                    nc.scalar.copy(kT_t[:, h, tsl], pk[h][:])
                for s4 in range(4):
                    half = s4 % 2
                    nc.scalar.copy(v_t[:, tt * 4 + s4, :],
                                   pv[s4 // 2][:, half * DC:(half + 1) * DC])
                # RoPE in place, all-SBUF bf16 (DVE 2x path).  Partition-
                # shifting is only legal on copy-class ops, so swap halves
                # with two copies; sinT rows 0:64 are pre-negated on the host
                # so one aligned multiply finishes rotate-half:
                #   rot = swap_halves(raw);  rot *= sinN;  dst = raw*cos + rot
                for dst_t in (qT_t, kT_t):
                    for h in range(HPC):
                        dst = dst_t[:, h, tsl]
                        rot = ropep.tile([P, 512], bf16, tag="rot")
                        nc.vector.tensor_copy(rot[0:64, :], dst[64:128, :])
                        nc.vector.tensor_copy(rot[64:128, :], dst[0:64, :])
                        nc.vector.tensor_mul(out=rot[:], in0=rot[:], in1=sin_t[:])
                        nc.vector.tensor_mul(out=dst, in0=dst, in1=cos_t[:])
                        nc.vector.tensor_add(out=dst, in0=dst, in1=rot[:])

            # ---- phase 2: attention + output projection ----
            pending_y = []

            def emit_yproj(onorm, b, qt):
                for s4 in range(4):
                    r0 = b * T + qt * 512 + s4 * P
                    ystage = ysp.tile([P, D], bf16, tag="ystage")
                    for dout in range(4):
                        py = ps.tile([P, 512], f32, tag="ps", name="py")
                        for h in range(HPC):
                            nc.tensor.matmul(
                                py[:],
                                onorm[:, h, s4 * P:(s4 + 1) * P],
                                wo_t[:, h, dout * 512:(dout + 1) * 512],
                                start=(h == 0), stop=(h == HPC - 1))
                        if dout % 2 == 0:
                            nc.scalar.copy(ystage[:, dout * 512:(dout + 1) * 512],
                                           py[:])
                        else:
                            nc.vector.tensor_copy(
                                ystage[:, dout * 512:(dout + 1) * 512], py[:])
                    nc.sync.dma_start(y[r0:r0 + P, :], ystage[:])

            def emit_attn(b, qt):
                qbase = b * T + qt * 512
                nkt = KT_PER_Q * (qt + 1)
                onorm = onp.tile([P, HPC, 512], bf16, tag="onorm")
                for h in range(HPC):
                    po = ps.tile([P, 512], f32, tag="ps", name="po")
                    pr = ps.tile([P, 512], f32, tag="ps", name="pr")

                    def emit_score(kt, h=h):
                        mi = kt - KT_PER_Q * qt  # >=0 on the diagonal
                        q0 = P * mi if mi > 0 else 0
                        free = 512 - q0
                        ksl = slice(b * T + kt * P, b * T + (kt + 1) * P)
                        pscore = ps.tile([P, 512], f32, tag="ps", name="pscore")
                        nc.tensor.matmul(pscore[:, :free], kT_t[:, h, ksl],
                                         qT_t[:, h, qbase + q0:qbase + 512],
                                         start=True, stop=True)
                        ptile = ptp.tile([P, 512], bf16, tag="pt", name="ptile")
                        nc.scalar.activation(ptile[:, :free], pscore[:, :free],
                                             mybir.ActivationFunctionType.Exp,
                                             scale=inv_sqrt_hd)
                        if mi >= 0:
                            nc.vector.tensor_mul(out=ptile[:, :free],
                                                 in0=ptile[:, :free],
                                                 in1=mask[:, :free])
                        return ptile, q0, free

                    # kt loop pipelined two deep so the PE has wait-free score
                    # work while the exp(+mask) chain of earlier kts completes.
                    tiles = {}
                    for kt in range(min(2, nkt)):
                        tiles[kt] = emit_score(kt)
                    for kt in range(nkt):
                        if kt + 2 < nkt:
                            tiles[kt + 2] = emit_score(kt + 2)
                        ptile, q0, free = tiles.pop(kt)
                        st, sp = (kt == 0), (kt == nkt - 1)
                        nc.tensor.matmul(po[:, q0:512],
                                         v_t[:, b * (T // P) + kt,
                                             h * HD:(h + 1) * HD],
                                         ptile[:, :free], start=st, stop=sp)
                        # row sums ride the PE too: ones[128,128]-stationary
                        # puts the per-query sums on every PSUM partition, so
                        # 1/r needs no partition broadcast.
                        nc.tensor.matmul(pr[:, q0:512], ones_sq[:],
                                         ptile[:, :free], start=st, stop=sp)
                    nc.scalar.copy(onorm[:, h, :], po[:])
                    rr = rrp.tile([P, 512], f32, tag="rr")
                    nc.vector.reciprocal_approx_fast(out=rr[:], in_=pr[:])
                    nc.vector.tensor_mul(out=onorm[:, h, :], in0=onorm[:, h, :],
                                         in1=rr[:])

                pending_y.append((onorm, b, qt))
                if len(pending_y) > 2:
                    emit_yproj(*pending_y.pop(0))

            # ---- schedule: interleave attention between projection tiles ----
            for tt in range(TT):
                emit_tile(tt)
                if tt == 1:
                    for h in range(HPC):
                        nc.scalar.dma_start(
                            wo_t[:, h, :],
                            woT.rearrange("(ko ki) n -> ki ko n", ki=P)[:, h, :])
                # attention unit (b,qt) is ready once tiles 0..(b*4+qt) exist
                if tt >= 1:
                    b, qt = divmod(tt - 1, QT)
                    emit_attn(b, qt)
            emit_attn(1, 3)
            for args in pending_y:
                emit_yproj(*args)

    nc.compile()
    return nc


def get_nc():
    if "nc" not in _CACHE:
        _CACHE["nc"] = _build_nc()
    return _CACHE["nc"]


def make_in_maps(x, cos, sin, wq, wk, wv, wo):
    bf = ml_dtypes.bfloat16
    xT = x.reshape(TOK, D).T  # [D, TOK]
    # [D, TOK] -> [TT, cpair, ci, j, 512]
    xTt = np.ascontiguousarray(
        xT.reshape(CPAIRS, 2, P, TT, 512).transpose(3, 0, 2, 1, 4)).astype(bf)
    cosT = np.ascontiguousarray(cos.reshape(TOK, HD).T).astype(bf)
    # rows 0:64 negated: rot_half contributes -x2*sin there (see kernel RoPE)
    sinT = np.ascontiguousarray(sin.reshape(TOK, HD).T).copy()
    sinT[0:64, :] *= -1.0
    sinT = sinT.astype(bf)
    in_maps = []
    for c in range(NCORES):
        dsl = slice(c * DC, (c + 1) * DC)
        in_maps.append({
            "xTt": xTt,
            "cosT": cosT,
            "sinT": sinT,
            "wqT": np.ascontiguousarray(wq[dsl, :].T).astype(bf),
            "wkT": np.ascontiguousarray(wk[dsl, :].T).astype(bf),
            "wvT": np.ascontiguousarray(wv[dsl, :].T).astype(bf),
            "woT": np.ascontiguousarray(wo[:, dsl].T).astype(bf),
        })
    return in_maps


def kernel(x, cos, sin, wq, wk, wv, wo):
    from concourse.bass_utils import run_bass_kernel_spmd

    nc = get_nc()
    in_maps = make_in_maps(
        np.asarray(x, dtype=np.float32), np.asarray(cos, dtype=np.float32),
        np.asarray(sin, dtype=np.float32), np.asarray(wq, dtype=np.float32),
        np.asarray(wk, dtype=np.float32), np.asarray(wv, dtype=np.float32),
        np.asarray(wo, dtype=np.float32))
    res = run_bass_kernel_spmd(nc, in_maps, list(range(NCORES)))
    out = np.zeros((TOK, D), dtype=np.float64)
    for m in res.results:
        out += m["y"].astype(np.float64)
    return out.astype(np.float32).reshape(B, T, D)


# revision 28
# speedup vs baseline: 1.6175x; 1.0105x over previous
"""Trainium2 Bass kernel for causal multi-head attention with RoPE.

Problem: x[2,2048,2048], 16 heads, head_dim 128, fp32.
  q/k/v = x @ w{q,k,v}^T ; RoPE on q,k ; causal softmax(q k^T / sqrt(128)) @ v ; out @ wo^T

Sharding: Megatron tensor-parallel over heads - 2 heads per core on 8 cores.
Each core computes a partial y (its 2 heads' contribution through wo); the host
sums the 8 partials.  No device collectives.

Per-core design (v2, all matmul operands bf16; fp8 was tested on CPU and
fails the 2e-2 gate at ~4e-2):
  - x pre-transposed/tiled bf16 on host; q^T,k^T computed feature-major,
    v token-major.  RoPE rotate-half built with a tiny constant matmul on
    the PE (prot = R^T q) so the DVE does only 3 tensor_tensor ops per
    RoPE application instead of 5.
  - scores computed transposed S^T[key,q] = kT.T @ qT, one K=128 pass.
    Causal handled at 128-granularity: for the 4 diagonal-crossing key
    tiles the query slice is trimmed to [128*mi : 512], which makes the
    score/exp/AV work exactly the lower-triangular block count; the
    remaining triangle uses a single [128,512] 0/1 bf16 mask (prefix
    slices of it serve every trim width).
  - softmax without max-subtraction (scores bounded, exp safe in fp32):
    P^T = exp(S^T/sqrt(128)) on ACT, bf16.
  - row sums: P tiles are accumulated into an f32 SBUF tile on the Pool
    engine (which is otherwise idle); one ones[128,128]-stationary matmul
    per (qt,h) then yields the per-query sums replicated across all 128
    PSUM partitions, so 1/r comes from one fast [128,512]
    reciprocal_approx_fast and feeds a plain tensor_tensor multiply - no
    partition_broadcast, no slow single-partition reciprocal.
  - o^T = v.T @ P^T accumulated in PSUM; normalization deferred by one
    half-unit so the PE never waits on the Pool accumulation.
  - y rows = (o_norm^T).T @ woT written bf16 (host sums partials in
    fp64); PSUM->SBUF y copies alternate ACT/DVE to balance engines.
  - phase interleaving: attention of (b,qt) is emitted as soon as its
    token tiles are projected, filling the projection-phase gaps.
"""

import math
import sys

sys.path.insert(0, "/opt/trn_rl_repo")

import ml_dtypes  # noqa: E402
import numpy as np  # noqa: E402

P = 128
D = 2048
HD = 128  # head dim
B = 2
T = 2048
TOK = B * T  # 4096
NCORES = 8
HPC = 2  # heads per core
DC = HPC * HD  # 256 dims per core
CCHUNKS = D // P  # 16 contraction chunks
CPAIRS = CCHUNKS // 2  # 8 chunk pairs (one DMA each)
TT = TOK // 512  # 8 token tiles of 512
QT = T // 512  # 4 query tiles per batch
KT_PER_Q = 512 // P  # 4 key tiles per query tile

_CACHE = {}


def _build_nc():
    import concourse.bacc as bacc
    import concourse.mybir as mybir
    import concourse.tile as tile

    f32 = mybir.dt.float32
    f32r = mybir.dt.float32r
    bf16 = mybir.dt.bfloat16

    nc = bacc.Bacc("TRN2", target_bir_lowering=False, debug=False, num_devices=NCORES)

    # x pre-tiled on host: [tt, cpair, 128, 2, 512] bf16, contiguous per pair
    xTt = nc.dram_tensor("xTt", [TT, CPAIRS, P, 2, 512], bf16,
                         kind="ExternalInput").ap()
    cosT = nc.dram_tensor("cosT", [HD, TOK], bf16, kind="ExternalInput").ap()
    sinT = nc.dram_tensor("sinT", [HD, TOK], bf16, kind="ExternalInput").ap()
    wqT = nc.dram_tensor("wqT", [D, DC], bf16, kind="ExternalInput").ap()
    wkT = nc.dram_tensor("wkT", [D, DC], bf16, kind="ExternalInput").ap()
    wvT = nc.dram_tensor("wvT", [D, DC], bf16, kind="ExternalInput").ap()
    woT = nc.dram_tensor("woT", [DC, D], bf16, kind="ExternalInput").ap()
    y = nc.dram_tensor("y", [TOK, D], bf16, kind="ExternalOutput").ap()

    inv_sqrt_hd = 1.0 / math.sqrt(HD)

    with tile.TileContext(nc) as tc:
        with (
            tc.tile_pool(name="consts", bufs=1) as consts,
            tc.tile_pool(name="wpool", bufs=1) as wpool,
            tc.tile_pool(name="qkv", bufs=1) as qkv,
            tc.tile_pool(name="xp", bufs=4) as xp,
            tc.tile_pool(name="csp", bufs=2) as csp,
            tc.tile_pool(name="ropep", bufs=2) as ropep,
            tc.tile_pool(name="ptp", bufs=4) as ptp,
            tc.tile_pool(name="rrp", bufs=2) as rrp,
            tc.tile_pool(name="onp", bufs=3) as onp,
            tc.tile_pool(name="ysp", bufs=3) as ysp,
            tc.tile_pool(name="ps", bufs=8, space="PSUM") as ps,
        ):
            # ---- constants ----
            # single causal 0/1 bf16 mask: keep where q_local - key_local >= 0.
            # Diagonal tile mi uses mask[:, :512-128*mi] against the trimmed
            # query slice starting at 128*mi.
            mask = consts.tile([P, 512], bf16, tag="mask")
            nc.gpsimd.memset(mask[:], 1.0)
            nc.gpsimd.affine_select(
                out=mask[:], in_=mask[:], compare_op=mybir.AluOpType.is_ge,
                fill=0.0, base=0, channel_multiplier=-1, pattern=[[1, 512]],
            )
            ones_sq = consts.tile([P, P], bf16, tag="ones_sq")
            nc.gpsimd.memset(ones_sq[:], 1.0)

            # ---- resident weights (DMAs staggered into tile 0's loop) ----
            wq_t = wpool.tile([P, CCHUNKS, DC], bf16, tag="wq")
            wk_t = wpool.tile([P, CCHUNKS, DC], bf16, tag="wk")
            wv_t = wpool.tile([P, CCHUNKS, DC], bf16, tag="wv")
            wo_t = wpool.tile([P, HPC, D], bf16, tag="wo")

            def emit_w_pair(cp):
                # weight traffic rides the (otherwise idle) Pool DGE queue so
                # it never delays the x-tile stream on the sync queue
                csl = slice(2 * cp, 2 * cp + 2)
                for wt, wdram in ((wq_t, wqT), (wk_t, wkT), (wv_t, wvT)):
                    nc.gpsimd.dma_start(
                        wt[:, csl, :],
                        wdram.rearrange("(co ci) d -> ci co d", ci=P)[:, csl, :])

            # ---- resident activations, one tile per 512-token block so the
            # dependency tracker keeps attention reads precise (a single big
            # tile accumulates 100+ writers and degrades to coarse deps that
            # serialize each attention unit behind the newest tile's RoPE) ----
            qT_ts = [qkv.tile([P, HPC, 512], bf16, tag=f"qT{t}", name=f"qT{t}")
                     for t in range(TT)]
            kT_ts = [qkv.tile([P, HPC, 512], bf16, tag=f"kT{t}", name=f"kT{t}")
                     for t in range(TT)]
            v_ts = [qkv.tile([P, 4, DC], bf16, tag=f"v{t}", name=f"v{t}")
                    for t in range(TT)]

            # ---- phase 1 tile body: projections + RoPE ----
            def emit_tile(tt):
                tsl = slice(tt * 512, (tt + 1) * 512)
                qT_t, kT_t, v_t = qT_ts[tt], kT_ts[tt], v_ts[tt]
                cos_t = csp.tile([P, 512], bf16, tag="cos")
                nc.scalar.dma_start(cos_t[:], cosT[:, tsl])
                sin_t = csp.tile([P, 512], bf16, tag="sin")
                nc.scalar.dma_start(sin_t[:], sinT[:, tsl])

                pq = [ps.tile([P, 512], f32, tag="ps", name=f"pq{i}") for i in range(HPC)]
                pk = [ps.tile([P, 512], f32, tag="ps", name=f"pk{i}") for i in range(HPC)]
                # two banks hold all four v accumulators ([t128, 256] pairs
                # side by side); see start/skip_group_check notes below.
                pv = [ps.tile([P, 512], f32, tag="ps", name=f"pv{i}") for i in range(2)]

                for cp in range(CPAIRS):
                    if tt == 0 and cp == 0:
                        emit_w_pair(0)
                        emit_w_pair(1)
                    if tt == 0 and cp + 2 < CPAIRS:
                        emit_w_pair(cp + 2)
                    xt = xp.tile([P, 2, 512], bf16, tag="x")
                    nc.sync.dma_start(xt[:], xTt[tt, cp])
                    for j in range(2):
                        c = 2 * cp + j
                        st, sp = (c == 0), (c == CCHUNKS - 1)
                        xj = xt[:, j, :]
                        for h in range(HPC):
                            dsl = slice(h * HD, (h + 1) * HD)
                            nc.tensor.matmul(pq[h][:], wq_t[:, c, dsl], xj,
                                             start=st, stop=sp)
                            nc.tensor.matmul(pk[h][:], wk_t[:, c, dsl], xj,
                                             start=st, stop=sp)
                        for s4 in range(4):
                            half = s4 % 2
                            nc.tensor.matmul(
                                pv[s4 // 2][:, half * DC:(half + 1) * DC],
                                xt[:, j, s4 * P:(s4 + 1) * P],
                                wv_t[:, c, :],
                                start=st and half == 0, stop=sp,
                                skip_group_check=half == 1)

                # evacuate PSUM: q/k/v raw copies on ACT
                for h in range(HPC):
                    nc.scalar.copy(qT_t[:, h, :], pq[h][:])
                    nc.scalar.copy(kT_t[:, h, :], pk[h][:])
                for s4 in range(4):
                    half = s4 % 2
                    nc.scalar.copy(v_t[:, s4, :],
                                   pv[s4 // 2][:, half * DC:(half + 1) * DC])
                # RoPE in place, all-SBUF bf16 (DVE 2x path).  Partition-
                # shifting is only legal on copy-class ops, so swap halves
                # with two copies; sinT rows 0:64 are pre-negated on the host
                # so one aligned multiply finishes rotate-half:
                #   rot = swap_halves(raw);  rot *= sinN;  dst = raw*cos + rot
                for dst_t in (qT_t, kT_t):
                    for h in range(HPC):
                        dst = dst_t[:, h, :]
                        rot = ropep.tile([P, 512], bf16, tag="rot")
                        nc.vector.tensor_copy(rot[0:64, :], dst[64:128, :])
                        nc.vector.tensor_copy(rot[64:128, :], dst[0:64, :])
                        nc.vector.tensor_mul(out=rot[:], in0=rot[:], in1=sin_t[:])
                        nc.vector.tensor_mul(out=dst, in0=dst, in1=cos_t[:])
                        nc.vector.tensor_add(out=dst, in0=dst, in1=rot[:])

            # ---- phase 2: attention + output projection ----
            pending_y = []

            def emit_yproj(onorm, b, qt):
                for s4 in range(4):
                    r0 = b * T + qt * 512 + s4 * P
                    ystage = ysp.tile([P, D], bf16, tag="ystage")
                    for dout in range(4):
                        py = ps.tile([P, 512], f32, tag="ps", name="py")
                        for h in range(HPC):
                            nc.tensor.matmul(
                                py[:],
                                onorm[:, h, s4 * P:(s4 + 1) * P],
                                wo_t[:, h, dout * 512:(dout + 1) * 512],
                                start=(h == 0), stop=(h == HPC - 1))
                        if dout % 2 == 0:
                            nc.scalar.copy(ystage[:, dout * 512:(dout + 1) * 512],
                                           py[:])
                        else:
                            nc.vector.tensor_copy(
                                ystage[:, dout * 512:(dout + 1) * 512], py[:])
                    nc.sync.dma_start(y[r0:r0 + P, :], ystage[:])

            def emit_attn(b, qt):
                qtile = qT_ts[b * QT + qt]
                nkt = KT_PER_Q * (qt + 1)
                onorm = onp.tile([P, HPC, 512], bf16, tag="onorm")
                for h in range(HPC):
                    po = ps.tile([P, 512], f32, tag="ps", name="po")
                    pr = ps.tile([P, 512], f32, tag="ps", name="pr")

                    def emit_score(kt, h=h):
                        mi = kt - KT_PER_Q * qt  # >=0 on the diagonal
                        q0 = P * mi if mi > 0 else 0
                        free = 512 - q0
                        kt_t = kT_ts[b * QT + kt // 4]
                        k0 = (kt % 4) * P
                        pscore = ps.tile([P, 512], f32, tag="ps", name="pscore")
                        nc.tensor.matmul(pscore[:, :free],
                                         kt_t[:, h, k0:k0 + P],
                                         qtile[:, h, q0:512],
                                         start=True, stop=True)
                        ptile = ptp.tile([P, 512], bf16, tag="pt", name="ptile")
                        nc.scalar.activation(ptile[:, :free], pscore[:, :free],
                                             mybir.ActivationFunctionType.Exp,
                                             scale=inv_sqrt_hd)
                        if mi >= 0:
                            nc.vector.tensor_mul(out=ptile[:, :free],
                                                 in0=ptile[:, :free],
                                                 in1=mask[:, :free])
                        return ptile, q0, free

                    # kt loop pipelined two deep so the PE has wait-free score
                    # work while the exp(+mask) chain of earlier kts completes.
                    tiles = {}
                    for kt in range(min(2, nkt)):
                        tiles[kt] = emit_score(kt)
                    for kt in range(nkt):
                        if kt + 2 < nkt:
                            tiles[kt + 2] = emit_score(kt + 2)
                        ptile, q0, free = tiles.pop(kt)
                        st, sp = (kt == 0), (kt == nkt - 1)
                        nc.tensor.matmul(po[:, q0:512],
                                         v_ts[b * QT + kt // 4][:, kt % 4,
                                             h * HD:(h + 1) * HD],
                                         ptile[:, :free], start=st, stop=sp)
                        # row sums ride the PE too: ones[128,128]-stationary
                        # puts the per-query sums on every PSUM partition, so
                        # 1/r needs no partition broadcast.
                        nc.tensor.matmul(pr[:, q0:512], ones_sq[:],
                                         ptile[:, :free], start=st, stop=sp)
                    nc.scalar.copy(onorm[:, h, :], po[:])
                    rr = rrp.tile([P, 512], f32, tag="rr")
                    nc.vector.reciprocal_approx_fast(out=rr[:], in_=pr[:])
                    nc.vector.tensor_mul(out=onorm[:, h, :], in0=onorm[:, h, :],
                                         in1=rr[:])

                pending_y.append((onorm, b, qt))
                if len(pending_y) > 2:
                    emit_yproj(*pending_y.pop(0))

            # ---- schedule: interleave attention between projection tiles ----
            for tt in range(TT):
                emit_tile(tt)
                if tt == 1:
                    for h in range(HPC):
                        nc.scalar.dma_start(
                            wo_t[:, h, :],
                            woT.rearrange("(ko ki) n -> ki ko n", ki=P)[:, h, :])
                # attention unit (b,qt) is ready once tiles 0..(b*4+qt) exist
                if tt >= 1:
                    b, qt = divmod(tt - 1, QT)
                    emit_attn(b, qt)
            emit_attn(1, 3)
            for args in pending_y:
                emit_yproj(*args)

    nc.compile()
    return nc


def get_nc():
    if "nc" not in _CACHE:
        _CACHE["nc"] = _build_nc()
    return _CACHE["nc"]


def make_in_maps(x, cos, sin, wq, wk, wv, wo):
    bf = ml_dtypes.bfloat16
    xT = x.reshape(TOK, D).T  # [D, TOK]
    # [D, TOK] -> [TT, cpair, ci, j, 512]
    xTt = np.ascontiguousarray(
        xT.reshape(CPAIRS, 2, P, TT, 512).transpose(3, 0, 2, 1, 4)).astype(bf)
    cosT = np.ascontiguousarray(cos.reshape(TOK, HD).T).astype(bf)
    # rows 0:64 negated: rot_half contributes -x2*sin there (see kernel RoPE)
    sinT = np.ascontiguousarray(sin.reshape(TOK, HD).T).copy()
    sinT[0:64, :] *= -1.0
    sinT = sinT.astype(bf)
    in_maps = []
    for c in range(NCORES):
        dsl = slice(c * DC, (c + 1) * DC)
        in_maps.append({
            "xTt": xTt,
            "cosT": cosT,
            "sinT": sinT,
            "wqT": np.ascontiguousarray(wq[dsl, :].T).astype(bf),
            "wkT": np.ascontiguousarray(wk[dsl, :].T).astype(bf),
            "wvT": np.ascontiguousarray(wv[dsl, :].T).astype(bf),
            "woT": np.ascontiguousarray(wo[:, dsl].T).astype(bf),
        })
    return in_maps


def kernel(x, cos, sin, wq, wk, wv, wo):
    from concourse.bass_utils import run_bass_kernel_spmd

    nc = get_nc()
    in_maps = make_in_maps(
        np.asarray(x, dtype=np.float32), np.asarray(cos, dtype=np.float32),
        np.asarray(sin, dtype=np.float32), np.asarray(wq, dtype=np.float32),
        np.asarray(wk, dtype=np.float32), np.asarray(wv, dtype=np.float32),
        np.asarray(wo, dtype=np.float32))
    res = run_bass_kernel_spmd(nc, in_maps, list(range(NCORES)))
    out = np.zeros((TOK, D), dtype=np.float64)
    for m in res.results:
        out += m["y"].astype(np.float64)
    return out.astype(np.float32).reshape(B, T, D)
